# revision 6
# baseline (speedup 1.0000x reference)
"""Trainium2 Bass kernel for nn_Classifier_22625887715977 (sparse_attention).

Contract: kernel(**inputs) takes FULL unsharded inputs (bs=32) and returns the
full [32, 75, 6] logits. Internally shards the batch over 8 NeuronCores
(4 episodes per core), with all weights replicated.

Math (per episode, exact reassociation of the reference):
  s        = leaky(ss @ Wm1 + bm1) @ Wm2 + bm2
  avg      = mean_n [bw | bsm]                      (per episode, ungated)
  gvis     = sigmoid(avg @ Wvis + bvis) + 1 ;  gsem = sigmoid(avg @ Wsem + bsem) + 1
  q        = sc @ Wq + s @ Wqs                              [5, 1024]
  scores   = ((q @ Wk^T) * gvis) @ bw^T + ((q @ Wks^T) * gsem) @ bsm^T   (/32, softmax)
  out      = ((attn @ bw) * gvis) @ Wv
  out2     = out @ Wfc + sc
  fake     = mean_w out2 ; protos = [sc; fake] ; logits = 10 * cos(qf, protos)

This avoids materializing the expanded per-(episode,way) base bank entirely.
"""

import numpy as np

BS = 32
NCORES = 8
EPC = BS // NCORES          # episodes per core
NW = 5
B20 = EPC * NW              # 20 batched support rows per core
FD = 1024
FDC = FD // 128             # 8
SEM = 300
SEMCH = [(0, 128), (128, 128), (256, 44)]   # (offset, size) chunks of 300
NB = 512
NBC = NB // 128             # 4
NQ = 75
NPROTO = NW + 1             # 6

_MODULE_CACHE = {}


def _build_module(temp: float):
    import concourse.bass as bass
    import concourse.mybir as mybir
    import concourse.tile as tile
    from concourse import bacc

    f32 = mybir.dt.float32
    f32r = mybir.dt.float32r
    AF = mybir.ActivationFunctionType
    ALU = mybir.AluOpType
    AX = mybir.AxisListType

    nc = bacc.Bacc("TRN2", target_bir_lowering=False, debug=False)

    # ---------------- DRAM I/O ----------------
    di = lambda name, shape: nc.dram_tensor(name, shape, f32, kind="ExternalInput")
    sc_d = di("support_center", [EPC, NW, FD])
    bw_d = di("base_weights", [EPC, NB, FD])
    ss_d = di("support_seman", [EPC, NW, SEM])
    bsm_d = di("base_seman", [EPC, NB, SEM])
    qf_d = di("query_feature", [EPC, NQ, FD])
    wm1_d = di("Wm1", [SEM, SEM])
    bm1_d = di("bm1", [SEM, 1])
    wm2_d = di("Wm2", [SEM, SEM])
    bm2_d = di("bm2", [SEM, 1])
    wvis_d = di("Wvis", [FD + SEM, FD])
    bvis_d = di("bvis", [1, FD])
    wsem_d = di("Wsem", [FD + SEM, SEM])
    bsem_d = di("bsem", [1, SEM])
    wq_d = di("Wq", [FD, FD])
    wk_d = di("Wk", [FD, FD])
    wv_d = di("Wv", [FD, FD])
    wqs_d = di("Wqs", [SEM, FD])
    wks_d = di("Wks", [SEM, FD])
    wfc_d = di("Wfc", [FD, FD])
    # host-provided data-independent constants
    ident_d = di("aux_ident", [128, 128])
    inv512_d = di("aux_inv512", [128, 1])
    one4_d = di("aux_one4", [1, EPC])
    fifths_d = di("aux_fifths", [B20, EPC])
    out_d = nc.dram_tensor("out", [EPC, NQ, NPROTO], f32, kind="ExternalOutput")

    with tile.TileContext(nc) as tc:
        with (
            tc.tile_pool(name="const", bufs=1) as cpool,
            tc.tile_pool(name="wres", bufs=1) as wres,
            tc.tile_pool(name="wstream", bufs=2) as wstr,
            tc.tile_pool(name="wtiles", bufs=3) as wtl,
            tc.tile_pool(name="banks", bufs=EPC) as bpool,
            tc.tile_pool(name="acts", bufs=1) as apool,
            tc.tile_pool(name="stage", bufs=1) as spool,
            tc.tile_pool(name="stage2", bufs=2) as spool2,
            tc.tile_pool(name="qfp", bufs=1) as qpool,
            tc.tile_pool(name="pt", bufs=2, space="PSUM") as pt,
            tc.tile_pool(name="pacc", bufs=2, space="PSUM") as pacc,
            tc.tile_pool(name="psmall", bufs=2, space="PSUM") as psm,
        ):
            # ---------------- constants ----------------
            ident = cpool.tile([128, 128], f32, tag="ident")
            nc.sync.dma_start(ident[:], ident_d.ap())
            inv512 = cpool.tile([128, 1], f32r, tag="inv512")
            nc.sync.dma_start(inv512[:], inv512_d.ap().bitcast(f32r))
            one4 = cpool.tile([1, EPC], f32r, tag="one4")
            nc.sync.dma_start(one4[:], one4_d.ap().bitcast(f32r))
            fifths = cpool.tile([B20, EPC], f32r, tag="fifths")
            nc.sync.dma_start(fifths[:], fifths_d.ap().bitcast(f32r))
            bias_rows = cpool.tile([1, FD], f32r, tag="bias_rows")
            nc.sync.dma_start(bias_rows[:], bvis_d.ap().bitcast(f32r))
            bias_rows_s = cpool.tile([1, SEM], f32r, tag="bias_rows_s")
            nc.sync.dma_start(bias_rows_s[:], bsem_d.ap().bitcast(f32r))
            # per-partition bias cols for the sMLP
            bm1T = cpool.tile([128, 3], f32, tag="bm1T")
            bm2T = cpool.tile([128, 3], f32, tag="bm2T")
            for c, (off, sz) in enumerate(SEMCH):
                nc.sync.dma_start(bm1T[0:sz, c : c + 1], bm1_d.ap()[off : off + sz, :])
                nc.sync.dma_start(bm2T[0:sz, c : c + 1], bm2_d.ap()[off : off + sz, :])

            # transpose helper: in_ [p<=128, f<=128] (SBUF, any dtype) -> PSUM [f, p]
            def ptranspose(in_ap):
                p = in_ap.partition_size()
                f = in_ap.free_size()
                t = pt.tile([128, 128], f32, tag="tr")
                nc.tensor.transpose(t[0:f, 0:p], in_ap.bitcast(f32), ident[0:p, 0:p])
                return t

            # ---------------- resident small weights (sMLP) ----------------
            wm1 = wres.tile([128, 3, SEM], f32, tag="wm1")
            wm2 = wres.tile([128, 3, SEM], f32, tag="wm2")
            for c, (off, sz) in enumerate(SEMCH):
                nc.sync.dma_start(wm1[0:sz, c, :], wm1_d.ap()[off : off + sz, :])
                nc.sync.dma_start(wm2[0:sz, c, :], wm2_d.ap()[off : off + sz, :])

            # ---------------- load support/seman, transposes ----------------
            sc_nat = apool.tile([B20, FD], f32, tag="sc_nat")
            nc.sync.dma_start(sc_nat[:], sc_d.ap().rearrange("e w d -> (e w) d"))
            ss_nat = apool.tile([B20, SEM], f32, tag="ss_nat")
            nc.sync.dma_start(ss_nat[:], ss_d.ap().rearrange("e w d -> (e w) d"))

            scT = apool.tile([128, FDC, B20], f32r, tag="scT")
            for dc in range(FDC):
                t = ptranspose(sc_nat[:, dc * 128 : (dc + 1) * 128])
                nc.vector.tensor_copy(scT[:, dc, :], t[0:128, 0:B20])
            ssT = apool.tile([128, 3, B20], f32, tag="ssT")
            for c, (off, sz) in enumerate(SEMCH):
                t = ptranspose(ss_nat[:, off : off + sz])
                nc.vector.tensor_copy(ssT[0:sz, c, :], t[0:sz, 0:B20])

            # ---------------- sMLP: sT = (leaky(ss@Wm1+bm1)@Wm2+bm2)^T ----------------
            # matmul(out, lhsT=[K, M], rhs=[K, N]) -> out [M, N]:
            # lhsT = Wm1 chunk [ksz, msz], rhs = ssT [ksz, 20] -> out [msz, 20]
            # (transposed layout, partitions = out features)
            h1T = apool.tile([128, 3, B20], f32, tag="h1T")
            for mc, (moff, msz) in enumerate(SEMCH):
                ph = psm.tile([128, B20], f32, tag="ps1")
                for kc, (koff, ksz) in enumerate(SEMCH):
                    nc.tensor.matmul(ph[0:msz, :], wm1[0:ksz, kc, moff : moff + msz],
                                     ssT[0:ksz, kc, :], start=(kc == 0), stop=(kc == 2))
                # leaky(x + b) = max(x + b, 0.1*(x + b))
                a1 = apool.tile([128, 3, B20], f32, tag="lk_a")
                nc.vector.tensor_scalar(a1[0:msz, mc, :], ph[0:msz, :], bm1T[0:msz, mc : mc + 1],
                                        0.1, op0=ALU.add, op1=ALU.mult)
                nc.vector.tensor_scalar(h1T[0:msz, mc, :], ph[0:msz, :], bm1T[0:msz, mc : mc + 1],
                                        None, op0=ALU.add)
                nc.vector.tensor_tensor(h1T[0:msz, mc, :], h1T[0:msz, mc, :], a1[0:msz, mc, :],
                                        op=ALU.max)
            sT = apool.tile([128, 3, B20], f32r, tag="sT")
            for mc, (moff, msz) in enumerate(SEMCH):
                ph = psm.tile([128, B20], f32, tag="ps1")
                for kc, (koff, ksz) in enumerate(SEMCH):
                    nc.tensor.matmul(ph[0:msz, :], wm2[0:ksz, kc, moff : moff + msz],
                                     h1T[0:ksz, kc, :], start=(kc == 0), stop=(kc == 2))
                nc.vector.tensor_scalar(sT[0:msz, mc, :], ph[0:msz, :], bm2T[0:msz, mc : mc + 1],
                                        None, op0=ALU.add)

            # ---------------- banks + avg (per episode) ----------------
            bw_nat = []
            bsm_nat = []
            avgvT = apool.tile([128, FDC, EPC], f32r, tag="avgvT")
            avgsT = apool.tile([128, 3, EPC], f32r, tag="avgsT")
            for e in range(EPC):
                bwt = bpool.tile([128, NBC, FD], f32r, tag="bw")
                nc.sync.dma_start(bwt[:], bw_d.ap()[e].rearrange("(c p) d -> p c d", p=128).bitcast(f32r))
                bw_nat.append(bwt)
                bst = bpool.tile([128, NBC, SEM], f32r, tag="bsm")
                nc.sync.dma_start(bst[:], bsm_d.ap()[e].rearrange("(c p) d -> p c d", p=128).bitcast(f32r))
                bsm_nat.append(bst)

                pa = pacc.tile([1, FD], f32, tag="pacc")
                for h in range(2):
                    for c in range(NBC):
                        nc.tensor.matmul(pa[:, h * 512 : (h + 1) * 512], inv512[:],
                                         bwt[:, c, h * 512 : (h + 1) * 512],
                                         start=(c == 0), stop=(c == NBC - 1))
                avg_nat = spool.tile([1, FD], f32, tag="avg_nat")
                nc.vector.tensor_copy(avg_nat[:], pa[:])
                for dc in range(FDC):
                    t = ptranspose(avg_nat[:, dc * 128 : (dc + 1) * 128])
                    nc.vector.tensor_copy(avgvT[:, dc, e : e + 1], t[0:128, 0:1])

                ps_ = psm.tile([1, SEM], f32, tag="ps1")
                for c in range(NBC):
                    nc.tensor.matmul(ps_[:], inv512[:], bst[:, c, :], start=(c == 0), stop=(c == NBC - 1))
                avgs_nat = spool.tile([1, SEM], f32, tag="avgs_nat")
                nc.vector.tensor_copy(avgs_nat[:], ps_[:])
                for c, (off, sz) in enumerate(SEMCH):
                    t = ptranspose(avgs_nat[:, off : off + sz])
                    nc.vector.tensor_copy(avgsT[0:sz, c, e : e + 1], t[0:sz, 0:1])

            # ---------------- gates ----------------
            # gate_pre_vis [EPC, FD] = avg @ Wvis + bvis  (bias via K=1 ones-row matmul)
            pg = pacc.tile([EPC, FD], f32, tag="pacc")
            n_k = FDC + 3 + 1
            ki = 0
            for dc in range(FDC):
                wchunk = wstr.tile([128, FD], f32r, tag="wstream")
                nc.sync.dma_start(wchunk[:], wvis_d.ap()[dc * 128 : (dc + 1) * 128, :].bitcast(f32r))
                for h in range(2):
                    nc.tensor.matmul(pg[:, h * 512 : (h + 1) * 512], avgvT[:, dc, :],
                                     wchunk[:, h * 512 : (h + 1) * 512],
                                     start=(ki == 0), stop=(ki == n_k - 1))
                ki += 1
            for c, (off, sz) in enumerate(SEMCH):
                wchunk = wstr.tile([128, FD], f32r, tag="wstream")
                nc.sync.dma_start(wchunk[0:sz, :], wvis_d.ap()[FD + off : FD + off + sz, :].bitcast(f32r))
                for h in range(2):
                    nc.tensor.matmul(pg[:, h * 512 : (h + 1) * 512], avgsT[0:sz, c, :],
                                     wchunk[0:sz, h * 512 : (h + 1) * 512],
                                     start=False, stop=(ki == n_k - 1))
                ki += 1
            for h in range(2):
                nc.tensor.matmul(pg[:, h * 512 : (h + 1) * 512], one4[:],
                                 bias_rows[:, h * 512 : (h + 1) * 512],
                                 start=False, stop=(ki == n_k - 1))
            ki += 1
            gpre_vis = spool.tile([EPC, FD], f32, tag="gpre")
            nc.vector.tensor_copy(gpre_vis[:], pg[:])

            pgs = psm.tile([EPC, SEM], f32, tag="ps1")
            ki = 0
            for dc in range(FDC):
                wchunk = wstr.tile([128, SEM], f32r, tag="wstream_s")
                nc.sync.dma_start(wchunk[0:128, 0:SEM], wsem_d.ap()[dc * 128 : (dc + 1) * 128, :].bitcast(f32r))
                nc.tensor.matmul(pgs[:], avgvT[:, dc, :], wchunk[0:128, 0:SEM],
                                 start=(ki == 0), stop=False)
                ki += 1
            for c, (off, sz) in enumerate(SEMCH):
                wchunk = wstr.tile([128, SEM], f32r, tag="wstream_s")
                nc.sync.dma_start(wchunk[0:sz, 0:SEM], wsem_d.ap()[FD + off : FD + off + sz, :].bitcast(f32r))
                nc.tensor.matmul(pgs[:], avgsT[0:sz, c, :], wchunk[0:sz, 0:SEM],
                                 start=False, stop=False)
            nc.tensor.matmul(pgs[:], one4[:], bias_rows_s[:], start=False, stop=True)
            gpre_sem = spool.tile([EPC, SEM], f32, tag="gpre_s")
            nc.vector.tensor_copy(gpre_sem[:], pgs[:])

            # transpose gates, sigmoid, +1
            gvisT = apool.tile([128, FDC, EPC], f32, tag="gvisT")
            for dc in range(FDC):
                t = ptranspose(gpre_vis[:, dc * 128 : (dc + 1) * 128])
                nc.scalar.activation(gvisT[:, dc, :], t[0:128, 0:EPC], AF.Sigmoid)
                nc.vector.tensor_scalar_add(gvisT[:, dc, :], gvisT[:, dc, :], 1.0)
            gsemT = apool.tile([128, 3, EPC], f32, tag="gsemT")
            for c, (off, sz) in enumerate(SEMCH):
                t = ptranspose(gpre_sem[:, off : off + sz])
                nc.scalar.activation(gsemT[0:sz, c, :], t[0:sz, 0:EPC], AF.Sigmoid)
                nc.vector.tensor_scalar_add(gsemT[0:sz, c, :], gsemT[0:sz, c, :], 1.0)

            # ---------------- q = sc @ Wq + s @ Wqs  (natural [20, 1024]) ----------------
            pq = pacc.tile([B20, FD], f32, tag="pacc")
            nk = FDC + 3
            ki = 0
            for dc in range(FDC):
                wchunk = wstr.tile([128, FD], f32r, tag="wstream")
                nc.sync.dma_start(wchunk[:], wq_d.ap()[dc * 128 : (dc + 1) * 128, :].bitcast(f32r))
                for h in range(2):
                    nc.tensor.matmul(pq[:, h * 512 : (h + 1) * 512], scT[:, dc, :],
                                     wchunk[:, h * 512 : (h + 1) * 512],
                                     start=(ki == 0), stop=(ki == nk - 1))
                ki += 1
            for c, (off, sz) in enumerate(SEMCH):
                wchunk = wstr.tile([128, FD], f32r, tag="wstream")
                nc.sync.dma_start(wchunk[0:sz, :], wqs_d.ap()[off : off + sz, :].bitcast(f32r))
                for h in range(2):
                    nc.tensor.matmul(pq[:, h * 512 : (h + 1) * 512], sT[0:sz, c, :],
                                     wchunk[0:sz, h * 512 : (h + 1) * 512],
                                     start=False, stop=(ki == nk - 1))
                ki += 1
            q_nat = spool.tile([B20, FD], f32, tag="q_nat")
            nc.vector.tensor_copy(q_nat[:], pq[:])
            qT = apool.tile([128, FDC, B20], f32r, tag="qT")
            for dc in range(FDC):
                t = ptranspose(q_nat[:, dc * 128 : (dc + 1) * 128])
                nc.vector.tensor_copy(qT[:, dc, :], t[0:128, 0:B20])

            # ---------------- t1 = q @ Wk^T (gated), t2 = q @ Wks^T (gated) ----------------
            pt1 = pacc.tile([B20, FD], f32, tag="pacc")
            for kc in range(FDC):
                # build WkT chunk [j-slice kc (128), d (1024)] by transposing Wk tiles
                wkTc = wstr.tile([128, FD], f32r, tag="wstream")
                for dc in range(FDC):
                    wtile = wtl.tile([128, 128], f32, tag="wtile")
                    nc.sync.dma_start(wtile[:], wk_d.ap()[dc * 128 : (dc + 1) * 128,
                                                          kc * 128 : (kc + 1) * 128])
                    t = ptranspose(wtile[:])
                    nc.scalar.copy(wkTc[:, dc * 128 : (dc + 1) * 128], t[0:128, 0:128])
                for h in range(2):
                    nc.tensor.matmul(pt1[:, h * 512 : (h + 1) * 512], qT[:, kc, :],
                                     wkTc[:, h * 512 : (h + 1) * 512],
                                     start=(kc == 0), stop=(kc == FDC - 1))
            t1_nat = spool.tile([B20, FD], f32, tag="q_nat")
            nc.vector.tensor_copy(t1_nat[:], pt1[:])
            t1gT = apool.tile([128, FDC, B20], f32r, tag="t1gT")
            for dc in range(FDC):
                t = ptranspose(t1_nat[:, dc * 128 : (dc + 1) * 128])
                for e in range(EPC):
                    nc.vector.tensor_scalar(t1gT[:, dc, e * NW : (e + 1) * NW],
                                            t[0:128, e * NW : (e + 1) * NW],
                                            gvisT[:, dc, e : e + 1], None, op0=ALU.mult)

            pt2 = psm.tile([B20, SEM], f32, tag="ps1")
            for kc in range(FDC):
                wksTc = wstr.tile([128, SEM], f32r, tag="wstream_s")
                for c, (off, sz) in enumerate(SEMCH):
                    wtile = wtl.tile([128, 128], f32, tag="wtile")
                    nc.sync.dma_start(wtile[0:sz, :], wks_d.ap()[off : off + sz,
                                                                 kc * 128 : (kc + 1) * 128])
                    t = ptranspose(wtile[0:sz, :])
                    nc.scalar.copy(wksTc[:, off : off + sz], t[0:128, 0:sz])
                nc.tensor.matmul(pt2[:], qT[:, kc, :], wksTc[:],
                                 start=(kc == 0), stop=(kc == FDC - 1))
            t2_nat = spool.tile([B20, SEM], f32, tag="t2_nat")
            nc.vector.tensor_copy(t2_nat[:], pt2[:])
            t2gT = apool.tile([128, 3, B20], f32r, tag="t2gT")
            for c, (off, sz) in enumerate(SEMCH):
                t = ptranspose(t2_nat[:, off : off + sz])
                for e in range(EPC):
                    nc.vector.tensor_scalar(t2gT[0:sz, c, e * NW : (e + 1) * NW],
                                            t[0:sz, e * NW : (e + 1) * NW],
                                            gsemT[0:sz, c, e : e + 1], None, op0=ALU.mult)

            # ---------------- per-episode attention + head ----------------
            ugT = apool.tile([128, FDC, B20], f32r, tag="ugT")
            for e in range(EPC):
                bwt = bw_nat[e]
                bst = bsm_nat[e]
                # scores [5, 512] accumulated over 8 vis chunks + 3 sem chunks
                psc = psm.tile([NW, NB], f32, tag="ps1")
                for dc in range(FDC):
                    stg = spool2.tile([128, NB], f32r, tag="bwT_st")
                    for c4 in range(NBC):
                        t = ptranspose(bwt[:, c4, dc * 128 : (dc + 1) * 128])
                        nc.vector.tensor_copy(stg[:, c4 * 128 : (c4 + 1) * 128], t[0:128, 0:128])
                    nc.tensor.matmul(psc[:], t1gT[:, dc, e * NW : (e + 1) * NW], stg[:],
                                     start=(dc == 0), stop=False)
                for c, (off, sz) in enumerate(SEMCH):
                    stg = spool2.tile([128, NB], f32r, tag="bsmT_st")
                    for c4 in range(NBC):
                        t = ptranspose(bst[:, c4, off : off + sz])
                        nc.vector.tensor_copy(stg[0:sz, c4 * 128 : (c4 + 1) * 128], t[0:sz, 0:128])
                    nc.tensor.matmul(psc[:], t2gT[0:sz, c, e * NW : (e + 1) * NW], stg[0:sz, :],
                                     start=False, stop=(c == 2))

                # softmax over 512 (scale 1/32, max-subtracted)
                mx = spool2.tile([NW, 1], f32, tag="mx")
                nc.vector.reduce_max(mx[:], psc[:], axis=AX.X)
                mxn = spool2.tile([NW, 1], f32, tag="mxn")
                nc.vector.tensor_scalar(mxn[:], mx[:], -1.0 / 32.0, None, op0=ALU.mult)
                attn = spool2.tile([NW, NB], f32, tag="attn")
                sm = spool2.tile([NW, 1], f32, tag="sm")
                nc.scalar.activation(attn[:], psc[:], AF.Exp, bias=mxn[:], scale=1.0 / 32.0,
                                     accum_out=sm[:])
                rs = spool2.tile([NW, 1], f32, tag="rs")
                nc.vector.reciprocal(rs[:], sm[:])
                nc.vector.tensor_scalar(attn[:], attn[:], rs[:], None, op0=ALU.mult)

                attnT = spool2.tile([128, NBC, NW], f32r, tag="attnT")
                for c4 in range(NBC):
                    t = ptranspose(attn[:, c4 * 128 : (c4 + 1) * 128])
                    nc.vector.tensor_copy(attnT[:, c4, :], t[0:128, 0:NW])

                # u = attn @ bw  [5, 1024]
                pu = pacc.tile([NW, FD], f32, tag="pacc")
                for h in range(2):
                    for c4 in range(NBC):
                        nc.tensor.matmul(pu[:, h * 512 : (h + 1) * 512], attnT[:, c4, :],
                                         bwt[:, c4, h * 512 : (h + 1) * 512],
                                         start=(c4 == 0), stop=(c4 == NBC - 1))
                u_nat = spool.tile([NW, FD], f32, tag="u_nat")
                nc.vector.tensor_copy(u_nat[:], pu[:])
                for dc in range(FDC):
                    t = ptranspose(u_nat[:, dc * 128 : (dc + 1) * 128])
                    nc.vector.tensor_scalar(ugT[:, dc, e * NW : (e + 1) * NW],
                                            t[0:128, 0:NW],
                                            gvisT[:, dc, e : e + 1], None, op0=ALU.mult)

            # ---------------- out = ug @ Wv ; out2 = out @ Wfc + sc ----------------
            po = pacc.tile([B20, FD], f32, tag="pacc")
            for dc in range(FDC):
                wchunk = wstr.tile([128, FD], f32r, tag="wstream")
                nc.sync.dma_start(wchunk[:], wv_d.ap()[dc * 128 : (dc + 1) * 128, :].bitcast(f32r))
                for h in range(2):
                    nc.tensor.matmul(po[:, h * 512 : (h + 1) * 512], ugT[:, dc, :],
                                     wchunk[:, h * 512 : (h + 1) * 512],
                                     start=(dc == 0), stop=(dc == FDC - 1))
            out_nat = spool.tile([B20, FD], f32, tag="q_nat")
            nc.vector.tensor_copy(out_nat[:], po[:])
            outT = apool.tile([128, FDC, B20], f32r, tag="outT")
            for dc in range(FDC):
                t = ptranspose(out_nat[:, dc * 128 : (dc + 1) * 128])
                nc.vector.tensor_copy(outT[:, dc, :], t[0:128, 0:B20])

            po2 = pacc.tile([B20, FD], f32, tag="pacc")
            for dc in range(FDC):
                wchunk = wstr.tile([128, FD], f32r, tag="wstream")
                nc.sync.dma_start(wchunk[:], wfc_d.ap()[dc * 128 : (dc + 1) * 128, :].bitcast(f32r))
                for h in range(2):
                    nc.tensor.matmul(po2[:, h * 512 : (h + 1) * 512], outT[:, dc, :],
                                     wchunk[:, h * 512 : (h + 1) * 512],
                                     start=(dc == 0), stop=(dc == FDC - 1))
            out2 = apool.tile([B20, FD], f32r, tag="out2")
            nc.vector.tensor_tensor(out2[:], po2[:], sc_nat[:], op=ALU.add)

            # ---------------- fake prototypes + normalize ----------------
            pf = pacc.tile([EPC, FD], f32, tag="pacc")
            for h in range(2):
                nc.tensor.matmul(pf[:, h * 512 : (h + 1) * 512], fifths[:],
                                 out2[:, h * 512 : (h + 1) * 512], start=True, stop=True)

            # normalize sc rows -> pn_sc ; fake rows -> pn_fake (all fp32)
            sq = qpool.tile([NQ, FD], f32, tag="sq")
            ssq = spool.tile([B20, 1], f32, tag="ssq")
            nc.scalar.activation(sq[0:B20, :], sc_nat[:], AF.Square, accum_out=ssq[:])
            rq = spool.tile([B20, 1], f32, tag="rq")
            nc.vector.reciprocal(rq[:], ssq[:])
            inv_sc = spool.tile([B20, 1], f32, tag="inv_sc")
            nc.scalar.activation(inv_sc[:], rq[:], AF.Sqrt)
            pn_sc = apool.tile([B20, FD], f32, tag="pn_sc")
            nc.vector.tensor_scalar(pn_sc[:], sc_nat[:], inv_sc[:], None, op0=ALU.mult)

            ssf = spool.tile([EPC, 1], f32, tag="ssf")
            nc.scalar.activation(sq[0:EPC, :], pf[:], AF.Square, accum_out=ssf[:])
            rf = spool.tile([EPC, 1], f32, tag="rf")
            nc.vector.reciprocal(rf[:], ssf[:])
            inv_f = spool.tile([EPC, 1], f32, tag="inv_f")
            nc.scalar.activation(inv_f[:], rf[:], AF.Sqrt)
            pn_fk = apool.tile([EPC, FD], f32, tag="pn_fk")
            nc.vector.tensor_scalar(pn_fk[:], pf[:], inv_f[:], None, op0=ALU.mult)

            # pnT [128, dc, 24] with e-major proto columns (6e+w, 6e+5)
            pnT = apool.tile([128, FDC, EPC * NPROTO], f32, tag="pnT")
            for dc in range(FDC):
                t = ptranspose(pn_sc[:, dc * 128 : (dc + 1) * 128])
                dst = pnT[:, dc, :].rearrange("p (e s) -> p e s", s=NPROTO)[:, :, 0:NW]
                src = t[0:128, 0:B20].rearrange("p (e w) -> p e w", w=NW)
                nc.vector.tensor_copy(dst, src)
                t2_ = ptranspose(pn_fk[:, dc * 128 : (dc + 1) * 128])
                dst2 = pnT[:, dc, :].rearrange("p (e s) -> p e s", s=NPROTO)[:, :, NW]
                nc.vector.tensor_copy(dst2, t2_[0:128, 0:EPC])

            # ---------------- queries: logits = temp * cos(qf, protos) ----------------
            for e in range(EPC):
                qf_nat = qpool.tile([NQ, FD], f32, tag="qf_nat")
                nc.sync.dma_start(qf_nat[:], qf_d.ap()[e])
                sq_ = qpool.tile([NQ, FD], f32, tag="sq")
                ssq_ = spool.tile([NQ, 1], f32, tag="ssq_q")
                nc.scalar.activation(sq_[:], qf_nat[:], AF.Square, accum_out=ssq_[:])
                rq_ = spool.tile([NQ, 1], f32, tag="rq_q")
                nc.vector.reciprocal(rq_[:], ssq_[:])
                s10 = spool.tile([NQ, 1], f32, tag="s10")
                nc.scalar.activation(s10[:], rq_[:], AF.Sqrt, scale=float(temp) * float(temp))
                qn10 = qpool.tile([NQ, FD], f32, tag="qn10")
                nc.vector.tensor_scalar(qn10[:], qf_nat[:], s10[:], None, op0=ALU.mult)
                qnT = qpool.tile([128, FDC, NQ], f32, tag="qnT")
                for dc in range(FDC):
                    t = ptranspose(qn10[:, dc * 128 : (dc + 1) * 128])
                    nc.vector.tensor_copy(qnT[:, dc, :], t[0:128, 0:NQ])
                pl = psm.tile([NQ, NPROTO], f32, tag="ps1")
                for dc in range(FDC):
                    nc.tensor.matmul(pl[:], qnT[:, dc, :],
                                     pnT[:, dc, e * NPROTO : (e + 1) * NPROTO],
                                     start=(dc == 0), stop=(dc == FDC - 1))
                lg = spool.tile([NQ, NPROTO], f32, tag="lg")
                nc.vector.tensor_copy(lg[:], pl[:])
                nc.sync.dma_start(out_d.ap()[e], lg[:])

    nc.finalize()
    return nc


def _aux_inputs():
    ident = np.eye(128, dtype=np.float32)
    inv512 = np.full((128, 1), 1.0 / 512.0, dtype=np.float32)
    one4 = np.ones((1, EPC), dtype=np.float32)
    fifths = np.zeros((B20, EPC), dtype=np.float32)
    for e in range(EPC):
        fifths[e * NW : (e + 1) * NW, e] = 1.0 / NW
    return {
        "aux_ident": ident,
        "aux_inv512": inv512,
        "aux_one4": one4,
        "aux_fifths": fifths,
    }


def kernel(**inputs):
    from concourse.bass_utils import run_bass_kernel_spmd

    temp = float(np.asarray(inputs["temp"]))
    key = ("v1", temp)
    if key not in _MODULE_CACHE:
        _MODULE_CACHE[key] = _build_module(temp)
    nc = _MODULE_CACHE[key]

    aux = _aux_inputs()
    per_ep = ["support_center", "base_weights", "support_seman", "base_seman", "query_feature"]
    weights = ["Wm1", "bm1", "Wm2", "bm2", "Wvis", "bvis", "Wsem", "bsem",
               "Wq", "Wk", "Wv", "Wqs", "Wks", "Wfc"]
    in_maps = []
    for c in range(NCORES):
        m = {}
        for k in per_ep:
            m[k] = np.ascontiguousarray(np.asarray(inputs[k])[c * EPC : (c + 1) * EPC])
        for k in weights:
            a = np.ascontiguousarray(np.asarray(inputs[k], dtype=np.float32))
            if k in ("bm1", "bm2"):
                a = a.reshape(SEM, 1)
            elif k == "bvis":
                a = a.reshape(1, FD)
            elif k == "bsem":
                a = a.reshape(1, SEM)
            m[k] = a
        m.update(aux)
        in_maps.append(m)

    res = run_bass_kernel_spmd(nc, in_maps, core_ids=list(range(NCORES)))
    out = np.concatenate([res.results[c]["out"] for c in range(NCORES)], axis=0)
    return out.astype(np.float32)


# revision 13
# speedup vs baseline: 1.2620x; 1.2620x over previous
"""Trainium2 Bass kernel for nn_Classifier_22625887715977 (sparse_attention).

kernel(**inputs) takes FULL unsharded inputs (bs=32) and returns the full
[32, 75, 6] logits. Shards the batch over 8 NeuronCores (4 episodes per
core); weights replicated and streamed.

Math (per episode, exact reassociation of the reference — never materializes
the expanded per-(episode,way) base bank):
  s      = leaky(ss @ Wm1 + bm1) @ Wm2 + bm2
  avg    = mean_n [bw | bsm]
  gvis   = sigmoid(avg @ Wvis + bvis) + 1 ; gsem = sigmoid(avg @ Wsem + bsem) + 1
  q      = sc @ Wq + s @ Wqs
  scores = ((q @ Wk^T) * gvis) @ bw^T + ((q @ Wks^T) * gsem) @ bsm^T ; attn = softmax(scores/32)
  out    = ((attn @ bw) * gvis) @ Wv ; out2 = out @ Wfc + sc
  fake   = mean_w out2 ; protos = [sc; fake] ; logits = temp * cos(qf, protos)

Implementation notes:
 - fp32r (full-rate fp32 matmul mode, ~1e-3 rel err) on the wide-N matmuls;
   exact fp32 on PE transposes and the final cosine path.
 - Transposed ("feature-on-partitions") layouts so episode packing happens on
   free dims (partition bases stay 32-aligned).
 - Three DMA queues: small loads on gpsimd (SWDGE), banks on sync (HWDGE/SP),
   weight streams on scalar (HWDGE/ACT), emitted at use sites.
 - PSUM accumulators are single-bank [*,512] halves.
"""

import numpy as np

BS = 32
NCORES = 8
EPC = BS // NCORES
NW = 5
B20 = EPC * NW
FD = 1024
FDC = FD // 128
SEM = 300
SEMCH = [(0, 128), (128, 128), (256, 44)]
NB = 512
NBC = NB // 128
NQ = 75
NPROTO = NW + 1

_MODULE_CACHE = {}


def _build_module(temp: float):
    import concourse.bass as bass
    import concourse.mybir as mybir
    import concourse.tile as tile
    from concourse import bacc

    f32 = mybir.dt.float32
    f32r = mybir.dt.float32r
    AF = mybir.ActivationFunctionType
    ALU = mybir.AluOpType
    AX = mybir.AxisListType

    nc = bacc.Bacc("TRN2", target_bir_lowering=False, debug=False)

    di = lambda name, shape: nc.dram_tensor(name, shape, f32, kind="ExternalInput")
    sc_d = di("support_center", [EPC, NW, FD])
    bw_d = di("base_weights", [EPC, NB, FD])
    ss_d = di("support_seman", [EPC, NW, SEM])
    bsm_d = di("base_seman", [EPC, NB, SEM])
    qf_d = di("query_feature", [EPC, NQ, FD])
    wm1_d = di("Wm1", [SEM, SEM])
    bm1_d = di("bm1", [SEM, 1])
    wm2_d = di("Wm2", [SEM, SEM])
    bm2_d = di("bm2", [SEM, 1])
    wvis_d = di("Wvis", [FD + SEM, FD])
    bvis_d = di("bvis", [1, FD])
    wsem_d = di("Wsem", [FD + SEM, SEM])
    bsem_d = di("bsem", [1, SEM])
    wq_d = di("Wq", [FD, FD])
    wk_d = di("Wk", [FD, FD])
    wv_d = di("Wv", [FD, FD])
    wqs_d = di("Wqs", [SEM, FD])
    wks_d = di("Wks", [SEM, FD])
    wfc_d = di("Wfc", [FD, FD])
    ident_d = di("aux_ident", [128, 128])
    inv512_d = di("aux_inv512", [128, 1])
    one4_d = di("aux_one4", [1, EPC])
    fifths_d = di("aux_fifths", [B20, EPC])
    out_d = nc.dram_tensor("out", [EPC, NQ, NPROTO], f32, kind="ExternalOutput")

    from contextlib import ExitStack
    with tile.TileContext(nc) as tc, ExitStack() as _ctx:
        def _pool(**kw):
            return _ctx.enter_context(tc.tile_pool(**kw))
        cpool = _pool(name="const", bufs=1)
        wres = _pool(name="wres", bufs=1)
        wbig = _pool(name="wbig", bufs=3)
        wktp = _pool(name="wkt", bufs=2)
        wlt = _pool(name="wlate", bufs=2)
        wsm = _pool(name="wsem", bufs=2)
        wkc = _pool(name="wkcol", bufs=2)
        bpool = _pool(name="banks", bufs=EPC)
        apool = _pool(name="acts", bufs=1)
        npool = _pool(name="nat4k", bufs=2)
        npool_s = _pool(name="nat12", bufs=2)
        sqpool = _pool(name="sq4k", bufs=2)
        qpool = _pool(name="qfp", bufs=2)
        qntp = _pool(name="qnt", bufs=4)
        smp = _pool(name="smalls", bufs=1)
        spool2 = _pool(name="stage2", bufs=2)
        pt = _pool(name="pt", bufs=3, space="PSUM")
        pacc = _pool(name="pacc", bufs=3, space="PSUM")
        psm = _pool(name="ps1", bufs=2, space="PSUM")
        if True:
            # ================= banks first on the sync queue =================
            bw_nat, bsm_nat = [], []
            for e in range(EPC):
                bwt = bpool.tile([128, NBC, FD], f32r, tag="bw")
                nc.sync.dma_start(bwt[:], bw_d.ap()[e].rearrange("(c p) d -> p c d", p=128).bitcast(f32r))
                bw_nat.append(bwt)
                bst = bpool.tile([128, NBC, SEM], f32r, tag="bsm")
                nc.sync.dma_start(bst[:], bsm_d.ap()[e].rearrange("(c p) d -> p c d", p=128).bitcast(f32r))
                bsm_nat.append(bst)

            # ================= small loads on the gpsimd (SWDGE) queue =======
            ident = cpool.tile([128, 128], f32, tag="ident")
            nc.gpsimd.dma_start(ident[:], ident_d.ap())
            inv512 = cpool.tile([128, 1], f32r, tag="inv512")
            nc.gpsimd.dma_start(inv512[:], inv512_d.ap().bitcast(f32r))
            one4 = cpool.tile([1, EPC], f32r, tag="one4")
            nc.gpsimd.dma_start(one4[:], one4_d.ap().bitcast(f32r))
            fifths = cpool.tile([B20, EPC], f32r, tag="fifths")
            nc.gpsimd.dma_start(fifths[:], fifths_d.ap().bitcast(f32r))
            bias_row_v = cpool.tile([1, FD], f32r, tag="bias_row_v")
            nc.gpsimd.dma_start(bias_row_v[:], bvis_d.ap().bitcast(f32r))
            bias_row_s = cpool.tile([1, SEM], f32r, tag="bias_row_s")
            nc.gpsimd.dma_start(bias_row_s[:], bsem_d.ap().bitcast(f32r))
            bm1T = cpool.tile([128, 3], f32, tag="bm1T")
            bm2T = cpool.tile([128, 3], f32, tag="bm2T")
            for c, (off, sz) in enumerate(SEMCH):
                nc.gpsimd.dma_start(bm1T[0:sz, c : c + 1], bm1_d.ap()[off : off + sz, :])
                nc.gpsimd.dma_start(bm2T[0:sz, c : c + 1], bm2_d.ap()[off : off + sz, :])
            sc_nat = apool.tile([B20, FD], f32, tag="sc_nat")
            nc.gpsimd.dma_start(sc_nat[:], sc_d.ap().rearrange("e w d -> (e w) d"))
            ss_nat = apool.tile([B20, SEM], f32, tag="ss_nat")
            nc.gpsimd.dma_start(ss_nat[:], ss_d.ap().rearrange("e w d -> (e w) d"))
            wm1 = wres.tile([128, 3, SEM], f32, tag="wm1")
            wm2 = wres.tile([128, 3, SEM], f32, tag="wm2")
            for c, (off, sz) in enumerate(SEMCH):
                nc.gpsimd.dma_start(wm1[0:sz, c, :], wm1_d.ap()[off : off + sz, :])
                nc.gpsimd.dma_start(wm2[0:sz, c, :], wm2_d.ap()[off : off + sz, :])
            qf_tiles = []
            for e in range(EPC):
                qt = qpool.tile([NQ, FD], f32, tag="qf_nat")
                nc.gpsimd.dma_start(qt[:], qf_d.ap()[e])
                qf_tiles.append(qt)

            # helpers
            def ptranspose(in_ap):
                p = in_ap.partition_size()
                f = in_ap.free_size()
                t = pt.tile([128, 128], f32, tag="tr")
                nc.tensor.transpose(t[0:f, 0:p], in_ap.bitcast(f32), ident[0:p, 0:p])
                return t

            _ci = [0]
            def copy_ps(dst, src):
                _ci[0] += 1
                if _ci[0] % 2:
                    nc.vector.tensor_copy(dst, src)
                else:
                    nc.scalar.copy(dst, src)

            # accumulate a [M,1024] = sum_k lhsT_k.T @ rhs_k via two 1-bank halves.
            # chunks: list of (lhsT_ap, rhs_full_ap) with rhs [K,1024]
            def acc_1024(m, chunks, out_cb):
                for h in range(2):
                    ph = pacc.tile([B20, 512], f32, tag="pacc")
                    n = len(chunks)
                    for i, (l, r) in enumerate(chunks):
                        nc.tensor.matmul(ph[0:m, :], l, r[:, h * 512 : (h + 1) * 512],
                                         start=(i == 0), stop=(i == n - 1))
                    out_cb(h, ph)

            # ================= sc/ss transposes + sMLP =================
            scT = apool.tile([128, FDC, B20], f32r, tag="scT")
            for dc in range(FDC):
                t = ptranspose(sc_nat[:, dc * 128 : (dc + 1) * 128])
                copy_ps(scT[:, dc, :], t[0:128, 0:B20])
            ssT = apool.tile([128, 3, B20], f32, tag="ssT")
            for c, (off, sz) in enumerate(SEMCH):
                t = ptranspose(ss_nat[:, off : off + sz])
                copy_ps(ssT[0:sz, c, :], t[0:sz, 0:B20])

            h1T = apool.tile([128, 3, B20], f32, tag="h1T")
            lk = apool.tile([128, 3, B20], f32, tag="lk")
            for mc, (moff, msz) in enumerate(SEMCH):
                ph = psm.tile([128, B20], f32, tag="ps1")
                for kc, (koff, ksz) in enumerate(SEMCH):
                    nc.tensor.matmul(ph[0:msz, :], wm1[0:ksz, kc, moff : moff + msz],
                                     ssT[0:ksz, kc, :], start=(kc == 0), stop=(kc == 2))
                nc.vector.tensor_scalar(lk[0:msz, mc, :], ph[0:msz, :], bm1T[0:msz, mc : mc + 1],
                                        0.1, op0=ALU.add, op1=ALU.mult)
                nc.vector.tensor_scalar(h1T[0:msz, mc, :], ph[0:msz, :], bm1T[0:msz, mc : mc + 1],
                                        None, op0=ALU.add)
                nc.vector.tensor_tensor(h1T[0:msz, mc, :], h1T[0:msz, mc, :], lk[0:msz, mc, :],
                                        op=ALU.max)
            sT = apool.tile([128, 3, B20], f32r, tag="sT")
            for mc, (moff, msz) in enumerate(SEMCH):
                ph = psm.tile([128, B20], f32, tag="ps1")
                for kc, (koff, ksz) in enumerate(SEMCH):
                    nc.tensor.matmul(ph[0:msz, :], wm2[0:ksz, kc, moff : moff + msz],
                                     h1T[0:ksz, kc, :], start=(kc == 0), stop=(kc == 2))
                nc.vector.tensor_scalar(sT[0:msz, mc, :], ph[0:msz, :], bm2T[0:msz, mc : mc + 1],
                                        None, op0=ALU.add)

            # ================= qf normalize + transpose (early) =================
            qnT_tiles = []
            for e in range(EPC):
                qt = qf_tiles[e]
                sq = sqpool.tile([NQ, FD], f32, tag="sq4k")
                ssq = smp.tile([NQ, 1], f32, tag="ssq_q")
                nc.scalar.activation(sq[:], qt[:], AF.Square, accum_out=ssq[:])
                rq = smp.tile([NQ, 1], f32, tag="rq_q")
                nc.vector.reciprocal(rq[:], ssq[:])
                s10 = smp.tile([NQ, 1], f32, tag="s10")
                nc.scalar.activation(s10[:], rq[:], AF.Sqrt, scale=float(temp) * float(temp))
                nc.vector.tensor_scalar(qt[:], qt[:], s10[:], None, op0=ALU.mult)
                qnT = qntp.tile([128, FDC, NQ], f32, tag="qnT")
                for dc in range(FDC):
                    t = ptranspose(qt[:, dc * 128 : (dc + 1) * 128])
                    copy_ps(qnT[:, dc, :], t[0:128, 0:NQ])
                qnT_tiles.append(qnT)

            # ================= q = sc @ Wq + s @ Wqs =================
            q_chunks = []
            for dc in range(FDC):
                w = wbig.tile([128, FD], f32r, tag="wbig")
                nc.scalar.dma_start(w[:], wq_d.ap()[dc * 128 : (dc + 1) * 128, :].bitcast(f32r))
                q_chunks.append((scT[:, dc, :], w[:, :]))
            for c, (off, sz) in enumerate(SEMCH):
                w = wbig.tile([128, FD], f32r, tag="wbig")
                nc.scalar.dma_start(w[0:sz, :], wqs_d.ap()[off : off + sz, :].bitcast(f32r))
                q_chunks.append((sT[0:sz, c, :], w[0:sz, :]))
            q_nat = npool.tile([B20, FD], f32, tag="nat4k")
            acc_1024(B20, q_chunks,
                     lambda h, ph: nc.vector.tensor_copy(q_nat[:, h * 512 : (h + 1) * 512], ph[0:B20, :]))
            qT = apool.tile([128, FDC, B20], f32r, tag="qT")
            for dc in range(FDC):
                t = ptranspose(q_nat[:, dc * 128 : (dc + 1) * 128])
                copy_ps(qT[:, dc, :], t[0:128, 0:B20])

            # ================= t1 = q @ Wk^T =================
            t1_chunks = []
            for kc in range(FDC):
                wt = wkc.tile([128, FDC, 128], f32, tag="wkcol")
                nc.scalar.dma_start(wt[:], wk_d.ap()[:, kc * 128 : (kc + 1) * 128]
                                    .rearrange("(c p) n -> p c n", p=128))
                wkTc = wktp.tile([128, FD], f32r, tag="wkT")
                for dc in range(FDC):
                    t = ptranspose(wt[:, dc, :])
                    copy_ps(wkTc[:, dc * 128 : (dc + 1) * 128], t[0:128, 0:128])
                t1_chunks.append((qT[:, kc, :], wkTc[:, :]))
            t1_nat = npool.tile([B20, FD], f32, tag="nat4k")
            acc_1024(B20, t1_chunks,
                     lambda h, ph: nc.vector.tensor_copy(t1_nat[:, h * 512 : (h + 1) * 512], ph[0:B20, :]))

            # ================= t2 = q @ Wks^T =================
            wks_nat = wres.tile([128, 3, FD], f32, tag="wks_nat")
            for c, (off, sz) in enumerate(SEMCH):
                nc.scalar.dma_start(wks_nat[0:sz, c, :], wks_d.ap()[off : off + sz, :])
            pt2 = psm.tile([B20, SEM], f32, tag="ps1")
            for kc in range(FDC):
                wksTc = wsm.tile([128, SEM], f32r, tag="wksT")
                for c, (off, sz) in enumerate(SEMCH):
                    t = ptranspose(wks_nat[0:sz, c, kc * 128 : (kc + 1) * 128])
                    copy_ps(wksTc[:, off : off + sz], t[0:128, 0:sz])
                nc.tensor.matmul(pt2[:], qT[:, kc, :], wksTc[:],
                                 start=(kc == 0), stop=(kc == FDC - 1))
            t2_nat = npool_s.tile([B20, SEM], f32, tag="nat12")
            nc.vector.tensor_copy(t2_nat[:], pt2[:])

            # ================= avg per episode =================
            avgvT = apool.tile([128, FDC, EPC], f32r, tag="avgvT")
            avgsT = apool.tile([128, 3, EPC], f32r, tag="avgsT")
            for e in range(EPC):
                avg_nat = npool.tile([1, FD], f32, tag="nat4k")
                acc_1024(1, [(inv512[:], bw_nat[e][:, c, :]) for c in range(NBC)],
                         lambda h, ph: nc.vector.tensor_copy(avg_nat[:, h * 512 : (h + 1) * 512], ph[0:1, :]))
                for dc in range(FDC):
                    t = ptranspose(avg_nat[:, dc * 128 : (dc + 1) * 128])
                    nc.vector.tensor_copy(avgvT[:, dc, e : e + 1], t[0:128, 0:1])
                ps_ = psm.tile([1, SEM], f32, tag="ps1")
                for c in range(NBC):
                    nc.tensor.matmul(ps_[:], inv512[:], bsm_nat[e][:, c, :],
                                     start=(c == 0), stop=(c == NBC - 1))
                avgs_nat = npool_s.tile([1, SEM], f32, tag="nat12")
                nc.vector.tensor_copy(avgs_nat[:], ps_[:])
                for c, (off, sz) in enumerate(SEMCH):
                    t = ptranspose(avgs_nat[:, off : off + sz])
                    nc.vector.tensor_copy(avgsT[0:sz, c, e : e + 1], t[0:sz, 0:1])

            # ================= gates =================
            g_chunks = []
            for dc in range(FDC):
                w = wbig.tile([128, FD], f32r, tag="wbig")
                nc.scalar.dma_start(w[:], wvis_d.ap()[dc * 128 : (dc + 1) * 128, :].bitcast(f32r))
                g_chunks.append((avgvT[:, dc, :], w[:, :]))
            for c, (off, sz) in enumerate(SEMCH):
                w = wbig.tile([128, FD], f32r, tag="wbig")
                nc.scalar.dma_start(w[0:sz, :], wvis_d.ap()[FD + off : FD + off + sz, :].bitcast(f32r))
                g_chunks.append((avgsT[0:sz, c, :], w[0:sz, :]))
            g_chunks.append((one4[:], bias_row_v[:, :]))
            gpre_vis = npool.tile([EPC, FD], f32, tag="nat4k")
            acc_1024(EPC, g_chunks,
                     lambda h, ph: nc.vector.tensor_copy(gpre_vis[:, h * 512 : (h + 1) * 512], ph[0:EPC, :]))

            pgs = psm.tile([EPC, SEM], f32, tag="ps1")
            wsem_list = []
            for dc in range(FDC):
                w = wsm.tile([128, SEM], f32r, tag="wsem")
                nc.scalar.dma_start(w[:], wsem_d.ap()[dc * 128 : (dc + 1) * 128, :].bitcast(f32r))
                wsem_list.append((avgvT[:, dc, :], w[0:128, :]))
            for c, (off, sz) in enumerate(SEMCH):
                w = wsm.tile([128, SEM], f32r, tag="wsem")
                nc.scalar.dma_start(w[0:sz, :], wsem_d.ap()[FD + off : FD + off + sz, :].bitcast(f32r))
                wsem_list.append((avgsT[0:sz, c, :], w[0:sz, :]))
            wsem_list.append((one4[:], bias_row_s[:, :]))
            for i, (l, r) in enumerate(wsem_list):
                nc.tensor.matmul(pgs[:], l, r, start=(i == 0), stop=(i == len(wsem_list) - 1))
            gpre_sem = npool_s.tile([EPC, SEM], f32, tag="nat12")
            nc.vector.tensor_copy(gpre_sem[:], pgs[:])

            gvisT = apool.tile([128, FDC, EPC], f32, tag="gvisT")
            for dc in range(FDC):
                t = ptranspose(gpre_vis[:, dc * 128 : (dc + 1) * 128])
                nc.scalar.activation(gvisT[:, dc, :], t[0:128, 0:EPC], AF.Sigmoid)
                nc.vector.tensor_scalar_add(gvisT[:, dc, :], gvisT[:, dc, :], 1.0)
            gsemT = apool.tile([128, 3, EPC], f32, tag="gsemT")
            for c, (off, sz) in enumerate(SEMCH):
                t = ptranspose(gpre_sem[:, off : off + sz])
                nc.scalar.activation(gsemT[0:sz, c, :], t[0:sz, 0:EPC], AF.Sigmoid)
                nc.vector.tensor_scalar_add(gsemT[0:sz, c, :], gsemT[0:sz, c, :], 1.0)

            # ================= gated projections t1g, t2g =================
            t1gT = apool.tile([128, FDC, B20], f32r, tag="t1gT")
            for dc in range(FDC):
                t = ptranspose(t1_nat[:, dc * 128 : (dc + 1) * 128])
                for e in range(EPC):
                    nc.vector.tensor_scalar(t1gT[:, dc, e * NW : (e + 1) * NW],
                                            t[0:128, e * NW : (e + 1) * NW],
                                            gvisT[:, dc, e : e + 1], None, op0=ALU.mult)
            t2gT = apool.tile([128, 3, B20], f32r, tag="t2gT")
            for c, (off, sz) in enumerate(SEMCH):
                t = ptranspose(t2_nat[:, off : off + sz])
                for e in range(EPC):
                    nc.vector.tensor_scalar(t2gT[0:sz, c, e * NW : (e + 1) * NW],
                                            t[0:sz, e * NW : (e + 1) * NW],
                                            gsemT[0:sz, c, e : e + 1], None, op0=ALU.mult)

            # ================= per-episode attention =================
            ugT = apool.tile([128, FDC, B20], f32r, tag="ugT")
            for e in range(EPC):
                bwt = bw_nat[e]
                bst = bsm_nat[e]
                psc = psm.tile([NW, NB], f32, tag="ps1")
                for dc in range(FDC):
                    stg = spool2.tile([128, NB], f32r, tag="bwT_st")
                    for c4 in range(NBC):
                        t = ptranspose(bwt[:, c4, dc * 128 : (dc + 1) * 128])
                        copy_ps(stg[:, c4 * 128 : (c4 + 1) * 128], t[0:128, 0:128])
                    nc.tensor.matmul(psc[:], t1gT[:, dc, e * NW : (e + 1) * NW], stg[:],
                                     start=(dc == 0), stop=False)
                for c, (off, sz) in enumerate(SEMCH):
                    stg = spool2.tile([128, NB], f32r, tag="bwT_st")
                    for c4 in range(NBC):
                        t = ptranspose(bst[:, c4, off : off + sz])
                        copy_ps(stg[0:sz, c4 * 128 : (c4 + 1) * 128], t[0:sz, 0:128])
                    nc.tensor.matmul(psc[:], t2gT[0:sz, c, e * NW : (e + 1) * NW], stg[0:sz, :],
                                     start=False, stop=(c == 2))

                mx = smp.tile([NW, 1], f32, tag="mx")
                nc.vector.reduce_max(mx[:], psc[:], axis=AX.X)
                mxn = smp.tile([NW, 1], f32, tag="mxn")
                nc.vector.tensor_scalar(mxn[:], mx[:], -1.0 / 32.0, None, op0=ALU.mult)
                attn = spool2.tile([NW, NB], f32, tag="attn")
                sm = smp.tile([NW, 1], f32, tag="sm")
                nc.scalar.activation(attn[:], psc[:], AF.Exp, bias=mxn[:], scale=1.0 / 32.0,
                                     accum_out=sm[:])
                rs = smp.tile([NW, 1], f32, tag="rs")
                nc.vector.reciprocal(rs[:], sm[:])
                nc.vector.tensor_scalar(attn[:], attn[:], rs[:], None, op0=ALU.mult)

                attnT = spool2.tile([128, NBC, NW], f32r, tag="attnT")
                for c4 in range(NBC):
                    t = ptranspose(attn[:, c4 * 128 : (c4 + 1) * 128])
                    copy_ps(attnT[:, c4, :], t[0:128, 0:NW])

                u_nat = npool.tile([NW, FD], f32, tag="nat4k")
                acc_1024(NW, [(attnT[:, c4, :], bwt[:, c4, :]) for c4 in range(NBC)],
                         lambda h, ph: nc.vector.tensor_copy(u_nat[:, h * 512 : (h + 1) * 512], ph[0:NW, :]))
                for dc in range(FDC):
                    t = ptranspose(u_nat[:, dc * 128 : (dc + 1) * 128])
                    nc.vector.tensor_scalar(ugT[:, dc, e * NW : (e + 1) * NW],
                                            t[0:128, 0:NW],
                                            gvisT[:, dc, e : e + 1], None, op0=ALU.mult)

            # ================= out = ug @ Wv ; out2 = out @ Wfc + sc =================
            o_chunks = []
            for dc in range(FDC):
                w = wlt.tile([128, FD], f32r, tag="wlate")
                nc.scalar.dma_start(w[:], wv_d.ap()[dc * 128 : (dc + 1) * 128, :].bitcast(f32r))
                o_chunks.append((ugT[:, dc, :], w[:, :]))
            out_nat = npool.tile([B20, FD], f32, tag="nat4k")
            acc_1024(B20, o_chunks,
                     lambda h, ph: nc.vector.tensor_copy(out_nat[:, h * 512 : (h + 1) * 512], ph[0:B20, :]))
            outT = apool.tile([128, FDC, B20], f32r, tag="outT")
            for dc in range(FDC):
                t = ptranspose(out_nat[:, dc * 128 : (dc + 1) * 128])
                copy_ps(outT[:, dc, :], t[0:128, 0:B20])

            o2_chunks = []
            for dc in range(FDC):
                w = wlt.tile([128, FD], f32r, tag="wlate")
                nc.scalar.dma_start(w[:], wfc_d.ap()[dc * 128 : (dc + 1) * 128, :].bitcast(f32r))
                o2_chunks.append((outT[:, dc, :], w[:, :]))
            out2 = apool.tile([B20, FD], f32r, tag="out2")
            acc_1024(B20, o2_chunks,
                     lambda h, ph: nc.vector.tensor_tensor(out2[:, h * 512 : (h + 1) * 512], ph[0:B20, :],
                                                           sc_nat[:, h * 512 : (h + 1) * 512], op=ALU.add))

            # ================= fake + normalize + pnT =================
            pn_fk = npool.tile([EPC, FD], f32, tag="nat4k")
            ssf = smp.tile([EPC, 1], f32, tag="ssf")
            sqp = sqpool.tile([NQ, FD], f32, tag="sq4k")

            def fake_half(h, ph):
                nc.vector.tensor_copy(pn_fk[:, h * 512 : (h + 1) * 512], ph[0:EPC, :])
            acc_1024(EPC, [(fifths[:], out2[:, :])], fake_half)

            ssq = smp.tile([B20, 1], f32, tag="ssq")
            nc.scalar.activation(sqp[0:B20, :], sc_nat[:], AF.Square, accum_out=ssq[:])
            rqv = smp.tile([B20, 1], f32, tag="rq")
            nc.vector.reciprocal(rqv[:], ssq[:])
            inv_sc = smp.tile([B20, 1], f32, tag="inv_sc")
            nc.scalar.activation(inv_sc[:], rqv[:], AF.Sqrt)
            pn_sc = sqpool.tile([B20, FD], f32, tag="sq4k")
            nc.vector.tensor_scalar(pn_sc[:], sc_nat[:], inv_sc[:], None, op0=ALU.mult)

            nc.scalar.activation(sqp[0:EPC, :], pn_fk[:], AF.Square, accum_out=ssf[:])
            rf = smp.tile([EPC, 1], f32, tag="rf")
            nc.vector.reciprocal(rf[:], ssf[:])
            inv_f = smp.tile([EPC, 1], f32, tag="inv_f")
            nc.scalar.activation(inv_f[:], rf[:], AF.Sqrt)
            nc.vector.tensor_scalar(pn_fk[:], pn_fk[:], inv_f[:], None, op0=ALU.mult)

            pnT = apool.tile([128, FDC, EPC * NPROTO], f32, tag="pnT")
            for dc in range(FDC):
                t = ptranspose(pn_sc[:, dc * 128 : (dc + 1) * 128])
                dst = pnT[:, dc, :].rearrange("p (e s) -> p e s", s=NPROTO)[:, :, 0:NW]
                src = t[0:128, 0:B20].rearrange("p (e w) -> p e w", w=NW)
                nc.vector.tensor_copy(dst, src)
                t2_ = ptranspose(pn_fk[:, dc * 128 : (dc + 1) * 128])
                dst2 = pnT[:, dc, :].rearrange("p (e s) -> p e s", s=NPROTO)[:, :, NW]
                nc.vector.tensor_copy(dst2, t2_[0:128, 0:EPC])

            # ================= logits =================
            for e in range(EPC):
                pl = psm.tile([NQ, NPROTO], f32, tag="ps1")
                for dc in range(FDC):
                    nc.tensor.matmul(pl[:], qnT_tiles[e][:, dc, :],
                                     pnT[:, dc, e * NPROTO : (e + 1) * NPROTO],
                                     start=(dc == 0), stop=(dc == FDC - 1))
                lg = smp.tile([NQ, NPROTO], f32, tag="lg")
                nc.vector.tensor_copy(lg[:], pl[:])
                nc.sync.dma_start(out_d.ap()[e], lg[:])

    nc.finalize()
    return nc


def _aux_inputs():
    ident = np.eye(128, dtype=np.float32)
    inv512 = np.full((128, 1), 1.0 / 512.0, dtype=np.float32)
    one4 = np.ones((1, EPC), dtype=np.float32)
    fifths = np.zeros((B20, EPC), dtype=np.float32)
    for e in range(EPC):
        fifths[e * NW : (e + 1) * NW, e] = 1.0 / NW
    return {
        "aux_ident": ident,
        "aux_inv512": inv512,
        "aux_one4": one4,
        "aux_fifths": fifths,
    }


def kernel(**inputs):
    from concourse.bass_utils import run_bass_kernel_spmd

    temp = float(np.asarray(inputs["temp"]))
    key = ("v3", temp)
    if key not in _MODULE_CACHE:
        _MODULE_CACHE[key] = _build_module(temp)
    nc = _MODULE_CACHE[key]

    aux = _aux_inputs()
    per_ep = ["support_center", "base_weights", "support_seman", "base_seman", "query_feature"]
    weights = ["Wm1", "bm1", "Wm2", "bm2", "Wvis", "bvis", "Wsem", "bsem",
               "Wq", "Wk", "Wv", "Wqs", "Wks", "Wfc"]
    in_maps = []
    for c in range(NCORES):
        m = {}
        for k in per_ep:
            m[k] = np.ascontiguousarray(np.asarray(inputs[k])[c * EPC : (c + 1) * EPC])
        for k in weights:
            a = np.ascontiguousarray(np.asarray(inputs[k], dtype=np.float32))
            if k in ("bm1", "bm2"):
                a = a.reshape(SEM, 1)
            elif k == "bvis":
                a = a.reshape(1, FD)
            elif k == "bsem":
                a = a.reshape(1, SEM)
            m[k] = a
        m.update(aux)
        in_maps.append(m)

    res = run_bass_kernel_spmd(nc, in_maps, core_ids=list(range(NCORES)))
    out = np.concatenate([res.results[c]["out"] for c in range(NCORES)], axis=0)
    return out.astype(np.float32)
